# revision 1
# baseline (speedup 1.0000x reference)
"""AttnBlock (GroupNorm + single-head 1x1-conv attention + residual) on 8 TRN2 cores.

Data-parallel over batch: core i processes x[i] (512, 64*64) entirely on-chip.

Math (per batch item, N = 64*64 = 4096 spatial positions, C = 512 channels):
  R = groupnorm(x)                          [C, N]
  scores = (Wq R)^T (Wk R) / sqrt(C) = R^T Ws R / sqrt(C),  Ws = Wq^T Wk (host)
  attn   = softmax(scores, axis=m)
  out    = x + Wp (V attn^T) + pb,  V = Wk R + kb
Key folds vs the straightforward version:
  - proj is folded into V on the host: V' = (Wp Wk) R, so the PV matmul output
    is directly the projected attention output; Wp kb folds into the bias
    (attn rows sum to 1), so no separate proj matmul and no fp8 re-quantization
    of the attention output are needed on device.
  - The softmax denominator d[n] is accumulated with an all-ones DoubleRow
    matmul (scores stay in the transposed [m, n] layout; no on-chip transposes)
    and 1/d is applied to the (linear) PV output at the end.
All big matmuls run in fp8e4m3 with perf_mode=DoubleRow (256-deep contraction
per pass); accumulation is fp32 in PSUM, GroupNorm statistics are fp32 (rsqrt
via bit-trick + 2 Newton steps on DVE so ScalarE only ever runs Exp -> no
activation-table swaps), and the final residual add is fp32 against a
freshly-DMA'd x tile.

Engine budget per batch item (cost-model):  PE ~139us (scores 55 + PV 55 +
d 14 + U/V' 14), ScalarE ~154us (exp of the 4096x4096 score matrix -- the
bottleneck), DVE ~100us (bn_stats, normalize, PSUM->SBUF fp8 copies, 1/d).
The rep loop is software-pipelined: GroupNorm of rep k+1 is emitted inside
rep k's attention loop at n-chunks 0..3 (DVE/DMA have slack there; the
PE-dependent steps are split across slots mt=16/18/24 so the tiny group-stat
matmuls never head-of-line-block the score stream), and U/V' of rep k+1 is
emitted one half-unit per slot at n-chunks 4..7 through a persistent
single-bank PSUM pool. PSUM budget: 2 score banks + 4 PV banks + 1 softmax-
denominator bank + 1 U/V' bank = 8. Two hardware-measured hazards shape the
schedule: a blocked 2-bank score ring degrades ~8x, so nothing that can lag
is emitted where it would stall PE mid-ring; and back-to-back accumulation
into the same PSUM bank is ~3x slower than the model, so the two DoubleRow
passes of adjacent score tiles (and of U/V' tiles) are interleaved to
alternate banks between consecutive matmuls.
"""
import sys

sys.path.insert(0, "/opt/trn_rl_repo")

import numpy as np
import ml_dtypes

import concourse.bass as bass
import concourse.bacc as bacc
import concourse.mybir as mybir
import concourse.tile as tile
from concourse import bass_utils

F32 = mybir.dt.float32
I32 = mybir.dt.int32
BF16 = mybir.dt.bfloat16
FP8 = mybir.dt.float8e4
DR = mybir.MatmulPerfMode.DoubleRow
AF = mybir.ActivationFunctionType
OP = mybir.AluOpType

B = 8
C = 512
N = 4096          # 64*64 spatial
GROUPS = 32
GSIZE = 16        # channels per group
EPS = 1e-6
CCH = 4           # channel chunks of 128
NCH = 8           # n chunks of 512
MT = 32           # m tiles of 128
P = 128
NW = 512          # matmul free dim / n-chunk width
NPAIR = MT // 2
INV_SQRT_C = 1.0 / float(np.sqrt(C))

_BUILD_CACHE = {}


def _build(use_amt: bool, use_kb: bool, reps: int = 1):
    nc = bacc.Bacc("TRN2", target_bir_lowering=False)

    x_in = nc.dram_tensor("x_in", [C, N], F32, kind="ExternalInput")
    wst_d = nc.dram_tensor("wst", [C, C], FP8, kind="ExternalInput")
    wvt_d = nc.dram_tensor("wvt", [C, C], FP8, kind="ExternalInput")
    gamma_d = nc.dram_tensor("gamma_r", [P, CCH], F32, kind="ExternalInput")
    beta_d = nc.dram_tensor("beta_r", [P, CCH], F32, kind="ExternalInput")
    pb_d = nc.dram_tensor("pb_r", [P, CCH], F32, kind="ExternalInput")
    ones_d = nc.dram_tensor("ones_b", [P, 2 * P], FP8, kind="ExternalInput")
    g_d = nc.dram_tensor("gmat", [P, 8], F32, kind="ExternalInput")
    g2_d = nc.dram_tensor("g2mat", [8, P], F32, kind="ExternalInput")
    if use_amt:
        amtw_d = nc.dram_tensor("amtw", [P, CCH], FP8, kind="ExternalInput")
    out_d = nc.dram_tensor("out", [C, N], F32, kind="ExternalOutput")

    with tile.TileContext(nc) as tc:
        # ---- persistent pools ----
        const = tc.alloc_tile_pool(name="const", bufs=1)
        xs_pool = tc.alloc_tile_pool(name="xs_pool", bufs=2)
        r8_pool = tc.alloc_tile_pool(name="r8_pool", bufs=2)
        u8_pool = tc.alloc_tile_pool(name="u8_pool", bufs=2)
        vt_pool = tc.alloc_tile_pool(name="vt_pool", bufs=2 * NPAIR)
        et_pool = tc.alloc_tile_pool(name="et_pool", bufs=8)
        xr_pool = tc.alloc_tile_pool(name="xr_pool", bufs=8)
        tt_pool = tc.alloc_tile_pool(name="tt_pool", bufs=4)
        ob_pool = tc.alloc_tile_pool(name="ob_pool", bufs=4)
        rd_pool = tc.alloc_tile_pool(name="rd_pool", bufs=2)
        bn_pool = tc.alloc_tile_pool(name="bn_pool", bufs=2)
        st_pool = tc.alloc_tile_pool(name="st_pool", bufs=2)
        # one persistent PSUM bank: U/V' staging + GN group-stat matmuls
        psv_pool = tc.alloc_tile_pool(name="psv_pool", bufs=1, space="PSUM")

        wst_sb = const.tile([P, CCH, NW], FP8)
        wvt_sb = const.tile([P, CCH, NW], FP8)
        gamma_sb = const.tile([P, CCH], F32)
        beta_sb = const.tile([P, CCH], F32)
        pb_sb = const.tile([P, CCH], F32)
        ones_sb = const.tile([P, 2, P], FP8)
        g_sb = const.tile([P, 8], F32)
        g2_sb = const.tile([8, P], F32)
        for cp in range(CCH):
            nc.sync.dma_start(out=wst_sb[:, cp, :], in_=wst_d[cp * P:(cp + 1) * P, :])
            nc.sync.dma_start(out=wvt_sb[:, cp, :], in_=wvt_d[cp * P:(cp + 1) * P, :])
        nc.sync.dma_start(out=gamma_sb, in_=gamma_d[:, :])
        nc.sync.dma_start(out=beta_sb, in_=beta_d[:, :])
        nc.sync.dma_start(out=pb_sb, in_=pb_d[:, :])
        nc.sync.dma_start(out=ones_sb, in_=ones_d[:, :].rearrange('p (a b) -> p a b', a=2))
        nc.sync.dma_start(out=g_sb, in_=g_d[:, :])
        nc.sync.dma_start(out=g2_sb, in_=g2_d[:, :])
        if use_amt:
            amtw_sb = const.tile([P, CCH, 1], FP8)
            nc.sync.dma_start(out=amtw_sb[:, :, 0], in_=amtw_d[:, :])
            amt_sb = const.tile([P, MT], F32)

        # -------- emission helpers (shared across pipeline stages) --------

        def emit_gn_dma(cp, x1):
            """DMA x chunk cp into the stream tile (issue well before use)."""
            for s in range(8):
                nc.sync.dma_start(out=x1[:, s * NW:(s + 1) * NW],
                                  in_=x_in[cp * P:(cp + 1) * P, s * NW:(s + 1) * NW])

        def emit_gn_chunk_front(cp, x1, r8_sb):
            """Per-partition bn stats of x chunk cp (DVE only)."""
            bnst = bn_pool.tile([P, 8, 6], F32, tag="bnst")
            for s in range(8):
                nc.vector.bn_stats(out=bnst[:, s, :], in_=x1[:, s * NW:(s + 1) * NW])
            mv = bn_pool.tile([P, 2], F32, tag="mv")
            nc.vector.bn_aggr(out=mv, in_=bnst)
            # S: per-partition [mean, E[x^2]]
            s_sb = bn_pool.tile([P, 2], F32, tag="s_sb")
            nc.vector.tensor_copy(out=s_sb[:, 0:1], in_=mv[:, 0:1])
            nc.vector.scalar_tensor_tensor(
                out=s_sb[:, 1:2], in0=mv[:, 0:1], scalar=mv[:, 0:1],
                in1=mv[:, 1:2], op0=OP.mult, op1=OP.add)
            return s_sb

        def emit_gn_back_a(s_sb):
            """Group-aggregate stats via PE (persistent psv bank)."""
            psg = psv_pool.tile([8, 2], F32, tag="psv")
            nc.tensor.matmul(psg, lhsT=g_sb, rhs=s_sb, start=True, stop=True)
            return psg

        def emit_gn_back_b(psg):
            """rsqrt chain on DVE (bit-trick + 2 Newton steps); returns the
            [rsqrt, mu] pair tile for the PE broadcast-back."""
            mu = st_pool.tile([8, 1], F32, tag="mu")
            nc.vector.tensor_scalar_mul(out=mu, in0=psg[:, 0:1], scalar1=1.0 / GSIZE)
            ex2 = st_pool.tile([8, 1], F32, tag="ex2")
            nc.vector.tensor_scalar_mul(out=ex2, in0=psg[:, 1:2], scalar1=1.0 / GSIZE)
            musq = st_pool.tile([8, 1], F32, tag="musq")
            nc.vector.tensor_mul(out=musq, in0=mu, in1=mu)
            veps = st_pool.tile([8, 1], F32, tag="veps")
            nc.vector.scalar_tensor_tensor(
                out=veps, in0=ex2, scalar=EPS, in1=musq, op0=OP.add, op1=OP.subtract)
            # rsqrt seed via the int bit-trick: y0 = bits(0x5F3759DF - (v >> 1))
            h_i = st_pool.tile([8, 1], I32, tag="h_i")
            nc.vector.tensor_scalar(
                out=h_i, in0=veps[:, :].bitcast(I32), scalar1=1, scalar2=None,
                op0=OP.arith_shift_right)
            y0_i = st_pool.tile([8, 1], I32, tag="y0_i")
            nc.vector.tensor_scalar(
                out=y0_i, in0=h_i, scalar1=-1, scalar2=0x5F3759DF,
                op0=OP.mult, op1=OP.add)
            # two Newton steps: y <- y * (1.5 - 0.5 * v * y^2)
            y = y0_i[:, :].bitcast(F32)
            for it in range(2):
                t1 = st_pool.tile([8, 1], F32, tag=f"t1_{it}")
                nc.vector.tensor_mul(out=t1, in0=y, in1=y)
                t2 = st_pool.tile([8, 1], F32, tag=f"t2_{it}")
                nc.vector.tensor_mul(out=t2, in0=t1, in1=veps)
                t3 = st_pool.tile([8, 1], F32, tag=f"t3_{it}")
                nc.vector.tensor_scalar(
                    out=t3, in0=t2, scalar1=-0.5, scalar2=1.5, op0=OP.mult, op1=OP.add)
                yn = st_pool.tile([8, 1], F32, tag=f"yn_{it}")
                nc.vector.tensor_mul(out=yn, in0=t3, in1=y)
                y = yn
            w_sb = st_pool.tile([8, 2], F32, tag="w_sb")
            nc.vector.tensor_copy(out=w_sb[:, 0:1], in_=y)
            nc.vector.tensor_copy(out=w_sb[:, 1:2], in_=mu)
            return w_sb

        def emit_gn_back_c(cp, x1, w_sb, r8_sb):
            """Broadcast group stats back via PE, then normalize x -> r8."""
            psp2 = psv_pool.tile([P, 2], F32, tag="psv")
            nc.tensor.matmul(psp2, lhsT=g2_sb, rhs=w_sb, start=True, stop=True)
            a_c = st_pool.tile([P, 1], F32, tag="a_c")
            nc.vector.tensor_mul(out=a_c, in0=gamma_sb[:, cp:cp + 1], in1=psp2[:, 0:1])
            tb = st_pool.tile([P, 1], F32, tag="tb")
            nc.vector.tensor_mul(out=tb, in0=psp2[:, 1:2], in1=a_c)
            b_c = st_pool.tile([P, 1], F32, tag="b_c")
            nc.vector.tensor_sub(out=b_c, in0=beta_sb[:, cp:cp + 1], in1=tb)
            nc.vector.tensor_scalar(out=r8_sb[:, cp, :], in0=x1,
                                    scalar1=a_c, scalar2=b_c,
                                    op0=OP.mult, op1=OP.add)

        def build_uv_units(r8_sb, u8_sb, vt_sb):
            """Return 64 closures, each emitting one U/V' tile: a DoubleRow
            matmul pair into the persistent psv bank plus a DVE fp8 copy out.
            The single-bank rotation paces PE behind the copies, which is fine
            because these are spread over attention slots with PE slack."""
            units = []

            def emit_u(cq, mc):
                st = {}

                def fa():
                    st["psv"] = psv_pool.tile([P, NW], F32, tag="psv",
                                              name="psv")
                    nc.tensor.matmul(
                        st["psv"],
                        lhsT=wst_sb[:, 0:2, cq * P:(cq + 1) * P],
                        rhs=r8_sb[:, 0:2, mc * NW:(mc + 1) * NW],
                        start=True, stop=False, perf_mode=DR)

                def fb():
                    nc.tensor.matmul(
                        st["psv"],
                        lhsT=wst_sb[:, 2:4, cq * P:(cq + 1) * P],
                        rhs=r8_sb[:, 2:4, mc * NW:(mc + 1) * NW],
                        start=False, stop=True, perf_mode=DR)
                    nc.vector.tensor_copy(
                        out=u8_sb[:, cq, mc * NW:(mc + 1) * NW], in_=st["psv"])
                return fa, fb

            def emit_v(mt):
                st = {}

                def fa():
                    if mt % 2 == 0:
                        vt_t = vt_pool.tile([P, 2, NW], FP8, tag="vt", name="vt")
                        vt_sb.append(vt_t)
                    st["psv"] = psv_pool.tile([P, NW], F32, tag="psv",
                                              name="psv")
                    nc.tensor.matmul(
                        st["psv"],
                        lhsT=r8_sb[:, 0:2, mt * P:(mt + 1) * P],
                        rhs=wvt_sb[:, 0:2, :],
                        start=True, stop=False, perf_mode=DR)

                def fb():
                    nc.tensor.matmul(
                        st["psv"],
                        lhsT=r8_sb[:, 2:4, mt * P:(mt + 1) * P],
                        rhs=wvt_sb[:, 2:4, :],
                        start=False, stop=True, perf_mode=DR)
                    nc.vector.tensor_copy(out=vt_sb[mt // 2][:, mt % 2, :],
                                          in_=st["psv"])
                    if use_amt:
                        psa = psv_pool.tile([P, 1], F32, tag="psv", name="psa")
                        for ks in (0, 2):
                            nc.tensor.matmul(
                                psa,
                                lhsT=r8_sb[:, ks:ks + 2, mt * P:(mt + 1) * P],
                                rhs=amtw_sb[:, ks:ks + 2, :],
                                start=(ks == 0), stop=(ks == 2), perf_mode=DR)
                        nc.vector.tensor_copy(out=amt_sb[:, mt:mt + 1], in_=psa)
                return fa, fb

            for mc in range(NCH):
                for cq in range(CCH):
                    units.extend(emit_u(cq, mc))
            for mt in range(MT):
                units.extend(emit_v(mt))
            return units

        # ================= main pipelined rep loop =================
        # Stage 3 of rep k carries, embedded in its emission, the work of rep
        # k+1 that must overlap with it: GroupNorm at nch 0..3 (front at
        # mt==6, back at mt==20 -- DVE/DMA have slack there) and the U/V'
        # production at nch 4..7 (one unit per even mt slot -- PE has slack
        # and the persistent psv bank paces it). Output drain of n-chunk
        # nch-1 sits at mt==1..4 so the pso banks free before this n-chunk's
        # first deferred PV needs them.

        def emit_stage3(r8_sb, u8_sb, vt_sb, next_rep):
            """Attention for one rep; emits GN + U/V' of next_rep (if not
            None) at fixed slots. Returns (r8, u8, vt) handles of next_rep."""
            nxt = {"r8": None, "u8": None, "vt": None, "x1": None, "s": None}
            uv_units = []
            if next_rep:
                nxt["r8"] = r8_pool.tile([P, CCH, N], FP8, tag="r8", name="r8")
                nxt["u8"] = u8_pool.tile([P, CCH, N], FP8, tag="u8", name="u8")
                nxt["vt"] = []
                uv_units = build_uv_units(nxt["r8"], nxt["u8"], nxt["vt"])

            with tc.tile_pool(name="pss", bufs=2, space="PSUM") as pss_pool, \
                 tc.tile_pool(name="pso", bufs=1, space="PSUM") as pso_pool, \
                 tc.tile_pool(name="psd", bufs=1, space="PSUM") as psd_pool:

                def emit_dpv(et_t, pt, psd_t, pso_tiles, first, last):
                    nc.tensor.matmul(psd_t, lhsT=ones_sb, rhs=et_t,
                                     start=first, stop=last, perf_mode=DR)
                    for cs in range(CCH):
                        nc.tensor.matmul(
                            pso_tiles[cs],
                            lhsT=vt_sb[pt][:, :, cs * P:(cs + 1) * P],
                            rhs=et_t, start=first, stop=last, perf_mode=DR)

                xr_tiles = {}

                def emit_out(state, cs):
                    pso_tiles, rd_t, pnch = state
                    t_t = tt_pool.tile([P, NW], F32, tag="t_t")
                    nc.vector.tensor_mul(out=t_t, in0=pso_tiles[cs], in1=rd_t)
                    ob = ob_pool.tile([P, NW], F32, tag="ob")
                    nc.vector.scalar_tensor_tensor(
                        out=ob, in0=t_t, scalar=pb_sb[:, cs:cs + 1],
                        in1=xr_tiles.pop((pnch, cs)),
                        op0=OP.add, op1=OP.add)
                    nc.sync.dma_start(
                        out=out_d[cs * P:(cs + 1) * P, pnch * NW:(pnch + 1) * NW],
                        in_=ob)

                state = None
                out_slots = {1: 0, 2: 1, 3: 2, 4: 3}
                xr_slots = {16: 0, 18: 1, 20: 2, 22: 3}
                DEPTH = 3
                for nch in range(NCH):
                    pso_tiles = [pso_pool.tile([P, NW], F32, tag=f"pso{cs}",
                                               name=f"pso{cs}") for cs in range(CCH)]
                    psd_t = psd_pool.tile([P, NW], F32, tag="psd")
                    pend = []
                    cur_et = None
                    pss_pair = [None, None]
                    for mt in range(MT):
                        half = mt % 2
                        if half == 0:
                            # both banks of the pair, ks-passes interleaved so
                            # consecutive matmuls never accumulate the same
                            # PSUM bank back-to-back (HW RMW hazard)
                            pss_pair[0] = pss_pool.tile([P, NW], F32,
                                                         tag="pss", name="pssA")
                            pss_pair[1] = pss_pool.tile([P, NW], F32,
                                                        tag="pss", name="pssB")
                            for ks in (0, 2):
                                for h in (0, 1):
                                    nc.tensor.matmul(
                                        pss_pair[h],
                                        lhsT=u8_sb[:, ks:ks + 2,
                                                   (mt + h) * P:(mt + h + 1) * P],
                                        rhs=r8_sb[:, ks:ks + 2,
                                                  nch * NW:(nch + 1) * NW],
                                        start=(ks == 0), stop=(ks == 2),
                                        perf_mode=DR)
                            cur_et = et_pool.tile([P, 2, NW], FP8, tag="et", name="et")
                        pss = pss_pair[half]
                        if use_amt:
                            nc.scalar.activation(out=cur_et[:, half, :], in_=pss,
                                                 func=AF.Exp, scale=INV_SQRT_C,
                                                 bias=amt_sb[:, mt:mt + 1])
                        else:
                            nc.scalar.activation(out=cur_et[:, half, :], in_=pss,
                                                 func=AF.Exp, scale=INV_SQRT_C)
                        if state is not None and mt in out_slots:
                            emit_out(state, out_slots[mt])
                        if mt in xr_slots:
                            cs = xr_slots[mt]
                            xr = xr_pool.tile([P, NW], F32, tag="xr")
                            nc.sync.dma_start(
                                out=xr,
                                in_=x_in[cs * P:(cs + 1) * P,
                                         nch * NW:(nch + 1) * NW])
                            xr_tiles[(nch, cs)] = xr
                        if mt == 6 and next_rep and nch < CCH:
                            nxt["x1"] = xs_pool.tile([P, N], F32, tag="x1", name="x1")
                            emit_gn_dma(nch, nxt["x1"])
                            nxt["s"] = emit_gn_chunk_front(nch, nxt["x1"], nxt["r8"])
                        if mt == 16 and next_rep and nch < CCH:
                            nxt["psg"] = emit_gn_back_a(nxt["s"])
                        if mt == 18 and next_rep and nch < CCH:
                            nxt["w"] = emit_gn_back_b(nxt["psg"])
                        if mt == 24 and next_rep and nch < CCH:
                            emit_gn_back_c(nch, nxt["x1"], nxt["w"], nxt["r8"])
                        if nch >= CCH and uv_units:
                            uv_units.pop(0)()
                        if mt % 2 == 1:
                            pend.append((cur_et, mt // 2))
                            if len(pend) > DEPTH:
                                p_et, pt = pend.pop(0)
                                emit_dpv(p_et, pt, psd_t, pso_tiles,
                                         first=(pt == 0), last=False)
                    for p_et, pt in pend:
                        emit_dpv(p_et, pt, psd_t, pso_tiles,
                                 first=(pt == 0), last=(pt == NPAIR - 1))
                    rd_t = rd_pool.tile([P, NW], F32, tag="rd")
                    nc.vector.reciprocal(out=rd_t, in_=psd_t)
                    state = (pso_tiles, rd_t, nch)
                # any units not placed (shouldn't happen: 64 units, 64 slots)
                for f in uv_units:
                    f()
                for cs in range(CCH):
                    emit_out(state, cs)
            return nxt

        # ---- prologue: GN + U/V of rep 0 (nothing to overlap with) ----
        r8_sb = r8_pool.tile([P, CCH, N], FP8, tag="r8", name="r8")
        for cp in range(CCH):
            x1 = xs_pool.tile([P, N], F32, tag="x1", name="x1")
            emit_gn_dma(cp, x1)
            s_sb = emit_gn_chunk_front(cp, x1, r8_sb)
            emit_gn_back_c(cp, x1, emit_gn_back_b(emit_gn_back_a(s_sb)), r8_sb)
        u8_sb = u8_pool.tile([P, CCH, N], FP8, tag="u8", name="u8")
        vt_sb = []
        for f in build_uv_units(r8_sb, u8_sb, vt_sb):
            f()

        for _rep in range(reps):
            nxt = emit_stage3(r8_sb, u8_sb, vt_sb, next_rep=(_rep + 1 < reps))
            r8_sb, u8_sb, vt_sb = nxt["r8"], nxt["u8"], nxt["vt"]

        for pool in (psv_pool, st_pool, bn_pool, rd_pool, ob_pool, tt_pool,
                     xr_pool, et_pool, vt_pool, u8_pool, r8_pool, xs_pool,
                     const):
            pool.release()

    nc.compile()
    return nc


def _prep_inputs(x, gn_gamma, gn_beta, q_w, q_b, k_w, k_b, proj_w, proj_b):
    use_amt = bool(np.any(q_b != 0))

    f8 = ml_dtypes.float8_e4m3
    f64 = np.float64
    ws_t = np.ascontiguousarray((k_w.T.astype(f64) @ q_w.astype(f64))
                                .astype(np.float32).astype(f8))
    # V' = (Wp Wk) R ; wvt = (Wp Wk)^T = Wk^T Wp^T
    wv_t = np.ascontiguousarray((k_w.T.astype(f64) @ proj_w.T.astype(f64))
                                .astype(np.float32).astype(f8))
    gamma_r = np.ascontiguousarray(gn_gamma.reshape(CCH, P).T.astype(np.float32))
    beta_r = np.ascontiguousarray(gn_beta.reshape(CCH, P).T.astype(np.float32))
    # attn rows sum to 1, so Wp kb is a constant channel bias: fold into pb
    pb_eff = (proj_b.astype(f64) + proj_w.astype(f64) @ k_b.astype(f64)).astype(
        np.float32)
    pb_r = np.ascontiguousarray(pb_eff.reshape(CCH, P).T)
    ones_b = np.ones((P, 2 * P), dtype=f8)
    gmat = np.zeros((P, 8), dtype=np.float32)
    gmat[np.arange(P), np.arange(P) // GSIZE] = 1.0
    g2mat = np.ascontiguousarray(gmat.T)

    common = {
        "wst": ws_t, "wvt": wv_t,
        "gamma_r": gamma_r, "beta_r": beta_r, "pb_r": pb_r,
        "ones_b": ones_b, "gmat": gmat, "g2mat": g2mat,
    }
    if use_amt:
        # amt[m] = qb . (Wk r_m) = (Wk^T qb) . r_m, pre-scaled by 1/sqrt(C);
        # the qb.kb term is constant over m AND n -> cancels in softmax.
        w_vec = (k_w.T.astype(f64) @ q_b.astype(f64)).astype(np.float32)
        w_vec = w_vec * INV_SQRT_C
        common["amtw"] = np.ascontiguousarray(
            w_vec.reshape(CCH, P).T.astype(f8))

    in_maps = []
    for i in range(B):
        m = dict(common)
        m["x_in"] = np.ascontiguousarray(x[i].reshape(C, N).astype(np.float32))
        in_maps.append(m)
    return in_maps, use_amt, False


def kernel(x, gn_gamma, gn_beta, q_w, q_b, k_w, k_b, proj_w, proj_b, _trace=False):
    x = np.asarray(x)
    in_maps, use_amt, use_kb = _prep_inputs(
        x, np.asarray(gn_gamma), np.asarray(gn_beta), np.asarray(q_w),
        np.asarray(q_b), np.asarray(k_w), np.asarray(k_b),
        np.asarray(proj_w), np.asarray(proj_b))

    key = (use_amt, use_kb)
    if key not in _BUILD_CACHE:
        _BUILD_CACHE[key] = _build(use_amt, use_kb)
    nc = _BUILD_CACHE[key]

    res = bass_utils.run_bass_kernel_spmd(
        nc, in_maps, core_ids=list(range(B)), trace=_trace)
    out = np.stack([r["out"].reshape(C, 64, 64) for r in res.results])
    kernel.last_result = res
    return out.astype(x.dtype)


def make_runner(inputs, chain=1):
    """Build the jitted 8-core executable once; return a callable that runs it
    once and returns wall ns, plus a decoder for the outputs."""
    import time
    import jax
    from jax.experimental.shard_map import shard_map
    from jax.sharding import Mesh, PartitionSpec
    from concourse import bass2jax
    import concourse.mybir as mb

    in_maps, use_amt, use_kb = _prep_inputs(
        np.asarray(inputs["x"]), np.asarray(inputs["gn_gamma"]),
        np.asarray(inputs["gn_beta"]), np.asarray(inputs["q_w"]),
        np.asarray(inputs["q_b"]), np.asarray(inputs["k_w"]),
        np.asarray(inputs["k_b"]), np.asarray(inputs["proj_w"]),
        np.asarray(inputs["proj_b"]))
    key = (use_amt, use_kb, chain)
    if key not in _BUILD_CACHE:
        _BUILD_CACHE[key] = _build(use_amt, use_kb, reps=chain)
    nc = _BUILD_CACHE[key]

    bass2jax.install_neuronx_cc_hook()
    partition_name = nc.partition_id_tensor.name if nc.partition_id_tensor else None
    in_names, out_names, out_avals, zero_outs = [], [], [], []
    for alloc in nc.m.functions[0].allocations:
        if not isinstance(alloc, mb.MemoryLocationSet):
            continue
        name = alloc.memorylocations[0].name
        if alloc.kind == "ExternalInput":
            if name != partition_name:
                in_names.append(name)
        elif alloc.kind == "ExternalOutput":
            out_names.append(name)
            shape = tuple(alloc.tensor_shape)
            dtype = mb.dt.np(alloc.dtype)
            out_avals.append(jax.core.ShapedArray(shape, dtype))
            zero_outs.append(np.zeros(shape, dtype))
    n_params = len(in_names)
    n_outs = len(out_avals)
    all_names = in_names + out_names
    if partition_name is not None:
        all_names = all_names + [partition_name]

    def _body(*args):
        operands = list(args)
        if partition_name is not None:
            operands.append(bass2jax.partition_id_tensor())
        outs = bass2jax._bass_exec_p.bind(
            *operands,
            out_avals=tuple(out_avals),
            in_names=tuple(all_names),
            out_names=tuple(out_names),
            lowering_input_output_aliases=(),
            sim_require_finite=True,
            sim_require_nnan=True,
            nc=nc,
        )
        return tuple(outs)

    donate = tuple(range(n_params, n_params + n_outs))
    devices = jax.devices()[:B]
    mesh = Mesh(np.asarray(devices), ("core",))
    sharded = jax.jit(
        shard_map(_body, mesh=mesh,
                  in_specs=(PartitionSpec("core"),) * (n_params + n_outs),
                  out_specs=(PartitionSpec("core"),) * n_outs,
                  check_rep=False),
        donate_argnums=donate, keep_unused=True)

    concat_in = [
        np.concatenate([np.asarray(in_maps[c][nm]) for c in range(B)], axis=0)
        for nm in in_names
    ]
    concat_zeros = [
        np.zeros((B * z.shape[0], *z.shape[1:]), z.dtype) for z in zero_outs
    ]
    sharding = jax.sharding.NamedSharding(mesh, PartitionSpec("core"))
    dev_in = [jax.device_put(a, sharding) for a in concat_in]

    state = {}

    def run_once():
        dev_zeros = [jax.device_put(z, sharding) for z in concat_zeros]
        for z in dev_zeros:
            z.block_until_ready()
        t0 = time.perf_counter()
        out_arrs = sharded(*dev_in, *dev_zeros)
        for o in out_arrs:
            o.block_until_ready()
        dt = (time.perf_counter() - t0) * 1e9
        state["out_arrs"] = out_arrs
        return dt

    def decode():
        out_arrs = state["out_arrs"]
        return [
            {nm: np.asarray(out_arrs[i]).reshape(B, *out_avals[i].shape)[c]
             for i, nm in enumerate(out_names)}
            for c in range(B)
        ]

    return run_once, decode


def bench(inputs, iters=6, chain=1):
    run_once, decode = make_runner(inputs, chain=chain)
    times = [run_once() for _ in range(iters)]
    return min(times), times, decode()

